# revision 1
# baseline (speedup 1.0000x reference)
"""Trainium2 Bass kernel for nn_Density_Softmax (retrieval_knn).

Math
----
reference() computes, for B=32, C=8192, D=256:

    confid[b,c,d] = density[b,d]/total_density[b,d] * (1-overly[b,c,d])
                    - density[b,d]/total_det[b,d] * overly[b,c,d]
    out = confid.mean()

with overly[b,c,d] = (c != argmin_c acd[b,:,d]) & (min2 - min1 >= 0.2*total_density[b,d])
(min1/min2 = two smallest of acd[b,:,d]; nontrivial is all-ones by construction).

Since min2 <= (S - min1)/(C-1)  (a minimum is <= the mean), the condition
min2 - min1 >= 0.2*S is impossible unless (S - C*min1)/(C-1) >= 0.2*max(S,1e-8).
For the graded inputs (acd uniform in [0,1), S ~ 4096) this never holds, so
overly == 0 identically and

    out = mean_{b,d}( density[b,d] / max(sum_c acd[b,c,d], 1e-8) ).

The kernel computes sum_c and min_c of acd on device (the 268 MB stream is
the memory-roofline cost), checks the bound exactly on host from (S, min),
and falls back to a full exact host implementation if the bound ever fails
or nontrivial is not all-ones.

Device layout (per core, 4 of 32 batch rows):
  - acd[b] is [8192, 256] row-major; a tile [128, KW] takes KW/256
    consecutive c-rows per partition => fully contiguous KW*4 bytes per
    partition per DMA (maximal DMA efficiency).
  - sum over c: PE matmul ones[128,1].T @ tile_chunk[128,512] accumulated in
    PSUM over all chunks/tiles of a batch row (fp32r: 1 cycle/row).
    Column f of the PSUM [1,512] result is the partial sum over d=f%256,
    c-rows with parity f//256; host adds the two halves.
  - min over c: one DVE tensor_reduce per tile over the AP [128, d=256, k]
    (innermost = the k c-rows in the partition), accumulated across tiles
    with an elementwise min; host min-reduces the 128 partition lanes.
"""

import os

import numpy as np

import concourse.bacc as bacc
import concourse.bass as bass
import concourse.tile as tile
from concourse import mybir
from concourse.bass_utils import run_bass_kernel_spmd

B, C, D = 32, 8192, 256
TOPK = 512
N_CORES = 8
BS = B // N_CORES  # batch rows per core

# Tile free width (f32 elements per partition). KW*512 bytes per DMA.
KW = int(os.environ.get("DS_KW", "4096"))
KW_MIN = int(os.environ.get("DS_KW_MIN", "512"))
N_BUFS = int(os.environ.get("DS_BUFS", "6"))
N_BUFS_R = int(os.environ.get("DS_BUFS_R", "5"))
# issue the first K input DMAs from GpSimd (SWDGE) so data flows while the
# sync engine is still in Tile's startup preamble
GPSIMD_HEAD = int(os.environ.get("DS_GPSIMD_HEAD", "0"))
# alternate input DMAs between the two HWDGE rings (sync / scalar issuing)
DMA_ALT = int(os.environ.get("DS_DMA_ALT", "0"))
# interleave tiles of batch rows 0..BS-2 round-robin (b-major when 0):
# spreads the HBM address stream across 4 regions and keeps 4 concurrent
# PSUM accumulation chains on the PE
INTERLEAVE = int(os.environ.get("DS_INTERLEAVE", "1"))

_FREE_PER_B = C * D // 128  # free elems per partition per batch row (16384)


def _tile_widths():
    """Per-batch-row tile widths. The last row tapers so the post-stream
    compute tail (ACT round + DVE reduce + PE matmuls on the final tile)
    is short. KW_MIN bounds the taper: very small DMAs cost more in fixed
    overhead than they save in tail."""
    base = [KW] * (_FREE_PER_B // KW)
    taper = [KW] * (_FREE_PER_B // KW - 1)
    rem = _FREE_PER_B - sum(taper)
    w = KW
    while rem > KW_MIN:
        w = max(KW_MIN, w // 2)
        taper.append(w)
        rem -= w
    if rem:
        taper.append(rem)
    widths = [list(base) for _ in range(BS - 1)] + [taper]
    assert all(sum(ws) == _FREE_PER_B for ws in widths)
    assert all(ww % 512 == 0 for ws in widths for ww in ws)
    return widths


T_TOTAL = sum(len(ws) for ws in _tile_widths())

_STATE = {}


def _build_nc():
    f32 = mybir.dt.float32
    f32r = mybir.dt.float32r
    nc = bacc.Bacc("TRN2", target_bir_lowering=False, debug=False)
    acd = nc.dram_tensor("acd", [BS, C, D], f32, kind="ExternalInput").ap()
    out_sum = nc.dram_tensor("out_sum", [BS, 512], f32, kind="ExternalOutput").ap()
    # per-tile global min over the whole tile (non-negativity certificate)
    out_min = nc.dram_tensor("out_min", [128, T_TOTAL], f32, kind="ExternalOutput").ap()
    # per-partition-lane partial sums of the last row's taper tiles (those
    # skip the ACT/PE path so the post-stream tail is DVE-only)
    out_part = nc.dram_tensor("out_part", [128, D], f32, kind="ExternalOutput").ap()

    with tile.TileContext(nc) as tc:
        with (
            tc.tile_pool(name="big", bufs=N_BUFS) as big_pool,
            tc.tile_pool(name="bigr", bufs=N_BUFS_R) as bigr_pool,
            tc.tile_pool(name="mins", bufs=1) as min_pool,
            tc.tile_pool(name="souts", bufs=2) as sum_pool,
            tc.tile_pool(name="const", bufs=1) as const_pool,
            tc.tile_pool(name="psum", bufs=4 if INTERLEAVE else 2, space="PSUM") as psum_pool,
        ):
            # fp32r ones for the PE partition-sum; memset can't write f32r,
            # and fp32r matmul operands must be produced by a rounding
            # instruction, so build via f32 memset + ACT cast.
            ones_f = const_pool.tile([128, 1], f32, tag="onesf")
            nc.vector.memset(ones_f[:], 1.0)
            ones = const_pool.tile([128, 1], f32r, tag="onesr")
            nc.scalar.copy(ones[:], ones_f[:])
            # one column per tile; host reduces to the global min
            mins = min_pool.tile([128, T_TOTAL], f32)
            widths = _tile_widths()
            # emission order: (b, t) pairs; optionally round-robin the
            # full-width rows so the HBM address stream is interleaved
            row_off = []
            for b in range(BS):
                offs, r0 = [], 0
                for kw in widths[b]:
                    offs.append(r0)
                    r0 += 128 * (kw // D)
                row_off.append(offs)
            n_full_last = sum(1 for w in widths[BS - 1] if w == KW)
            if INTERLEAVE:
                # round-robin ALL rows' full-width tiles; only the last
                # row's tapered tiles stay at the end (short compute tail)
                order = []
                for t in range(len(widths[0])):
                    for b in range(BS - 1):
                        order.append((b, t))
                    if t < n_full_last:
                        order.append((BS - 1, t))
                order += [
                    (BS - 1, t)
                    for t in range(n_full_last, len(widths[BS - 1]))
                ]
            else:
                order = [(b, t) for b in range(BS) for t in range(len(widths[b]))]
            col_of = {bt: i for i, bt in enumerate(order)}
            n_taper = len(widths[BS - 1]) - n_full_last
            n_full_total = len(order) - n_taper
            ps_of = {}
            spart = None
            for b, t in order:
                kw = widths[b][t]
                n_tiles = len(widths[b])
                tcol = col_of[(b, t)]
                if t == 0:
                    ps_of[b] = psum_pool.tile([1, 512], f32, name="ps", tag="ps")
                ps = ps_of[b]
                k_rows = kw // D
                row0 = row_off[b][t]
                src = acd[b, row0 : row0 + 128 * k_rows, :].rearrange(
                    "(p k) d -> p (k d)", p=128
                )
                # (DVE-only taper route measured slower: longer serial DVE
                # chain + extended big-slot lifetimes. Keep ACT/PE for all.)
                is_taper = False
                pe_last = n_tiles - 1
                big = big_pool.tile([128, kw], f32, tag="big")
                if tcol < GPSIMD_HEAD:
                    dma_eng = nc.gpsimd
                elif DMA_ALT and tcol % 2 == 1:
                    dma_eng = nc.scalar
                else:
                    dma_eng = nc.sync
                dma_eng.dma_start(big[:], src)
                if not is_taper:
                    # ACT rounding pass: fp32 -> fp32r (required by the PE
                    # fp32r datapath; ACT is otherwise idle)
                    bigr = bigr_pool.tile([128, kw], f32r, tag="bigr")
                    nc.scalar.copy(bigr[:], big[:])
                    for m in range(kw // 512):
                        nc.tensor.matmul(
                            ps[:],
                            ones[:],
                            bigr[:, bass.ts(m, 512)],
                            start=(t == 0 and m == 0),
                            stop=(t == pe_last and m == kw // 512 - 1),
                        )
                    nc.vector.tensor_reduce(
                        mins[:, tcol : tcol + 1], bigr[:].bitcast(f32),
                        axis=mybir.AxisListType.X, op=mybir.AluOpType.min,
                    )
                else:
                    # taper tiles: DVE-only (min + fold-add partial sums) so
                    # the post-stream tail avoids the ACT->PE->PSUM chain
                    nc.vector.tensor_reduce(
                        mins[:, tcol : tcol + 1], big[:],
                        axis=mybir.AxisListType.X, op=mybir.AluOpType.min,
                    )
                    w = kw // 2
                    while w >= D:
                        nc.vector.tensor_tensor(
                            big[:, :w], big[:, :w], big[:, w : 2 * w],
                            mybir.AluOpType.add,
                        )
                        w //= 2
                    if spart is None:
                        spart = min_pool.tile([128, D], f32, name="spart", tag="spart")
                        nc.vector.tensor_copy(spart[:], big[:, :D])
                    else:
                        nc.vector.tensor_tensor(
                            spart[:], spart[:], big[:, :D], mybir.AluOpType.add
                        )
                if t == pe_last:
                    sout = sum_pool.tile([1, 512], f32)
                    nc.scalar.copy(sout[:], ps[:])
                    nc.sync.dma_start(out_sum[b : b + 1, :], sout[:])
            # (splitting the mins DMA into an early full-tile piece measured
            # no better — the single end-of-kernel DMA is kept)
            nc.sync.dma_start(out_min, mins[:])
            if spart is not None:
                nc.sync.dma_start(out_part, spart[:])
    nc.compile()
    return nc


def _get_nc():
    if "nc" not in _STATE:
        _STATE["nc"] = _build_nc()
    return _STATE["nc"]


def _get_runner():
    """Sharded executor built once. Unlike bass2jax.run_bass_via_pjrt, the
    input shards are device_put and blocked-on BEFORE dispatch, so all 8
    cores start aligned and the kernel's HBM reads don't contend with
    input-upload writes."""
    if "runner" in _STATE:
        return _STATE["runner"]
    import jax
    import numpy as _np
    from jax.experimental.shard_map import shard_map
    from jax.sharding import Mesh, NamedSharding, PartitionSpec

    from concourse import bass2jax, mybir as _mybir

    bass2jax.install_neuronx_cc_hook()
    nc = _get_nc()

    partition_name = nc.partition_id_tensor.name if nc.partition_id_tensor else None
    in_names, out_names, out_avals, zero_outs = [], [], [], []
    for alloc in nc.m.functions[0].allocations:
        if not isinstance(alloc, _mybir.MemoryLocationSet):
            continue
        name = alloc.memorylocations[0].name
        if alloc.kind == "ExternalInput":
            if name != partition_name:
                in_names.append(name)
        elif alloc.kind == "ExternalOutput":
            out_names.append(name)
            shape = tuple(alloc.tensor_shape)
            dtype = _mybir.dt.np(alloc.dtype)
            out_avals.append(jax.core.ShapedArray(shape, dtype))
            zero_outs.append(_np.zeros(shape, dtype))
    n_params = len(in_names)
    n_outs = len(out_avals)
    all_in_names = list(in_names) + list(out_names)
    if partition_name is not None:
        all_in_names.append(partition_name)
    donate = tuple(range(n_params, n_params + n_outs))

    def _body(*args):
        operands = list(args)
        if partition_name is not None:
            operands.append(bass2jax.partition_id_tensor())
        outs = bass2jax._bass_exec_p.bind(
            *operands,
            out_avals=tuple(out_avals),
            in_names=tuple(all_in_names),
            out_names=tuple(out_names),
            lowering_input_output_aliases=(),
            sim_require_finite=True,
            sim_require_nnan=True,
            nc=nc,
        )
        return tuple(outs)

    devices = jax.devices()[:N_CORES]
    mesh = Mesh(_np.asarray(devices), ("core",))
    spec = NamedSharding(mesh, PartitionSpec("core"))
    in_specs = (PartitionSpec("core"),) * (n_params + n_outs)
    out_specs = (PartitionSpec("core"),) * n_outs
    sharded = jax.jit(
        shard_map(_body, mesh=mesh, in_specs=in_specs, out_specs=out_specs,
                  check_rep=False),
        donate_argnums=donate,
        keep_unused=True,
    )

    def run(in_map_global):
        import jax as _jax

        args = []
        for name in in_names:
            args.append(_jax.device_put(in_map_global[name], spec))
        for z in zero_outs:
            gz = _np.zeros((N_CORES * z.shape[0], *z.shape[1:]), z.dtype)
            args.append(_jax.device_put(gz, spec))
        for a in args:
            a.block_until_ready()
        outs = sharded(*args)
        outs = [_np.asarray(o) for o in outs]
        return [
            {
                name: outs[i].reshape(N_CORES, *out_avals[i].shape)[c]
                for i, name in enumerate(out_names)
            }
            for c in range(N_CORES)
        ]

    _STATE["runner"] = run
    return run


class _Res:
    def __init__(self, results):
        self.results = results


def _run_device(acd, **kw):
    try:
        return _Res(_get_runner()({"acd": acd}))
    except Exception:
        # robust fallback: stock SPMD path (handles native-NRT and axon)
        nc = _get_nc()
        in_maps = [{"acd": acd[i * BS : (i + 1) * BS]} for i in range(N_CORES)]
        return run_bass_kernel_spmd(nc, in_maps, list(range(N_CORES)))


def _reference_host(weight, mu, var, acd, labels, nontrivial):
    """Exact numpy mirror of reference.py (fallback; not used for graded
    inputs, where the overly mask is provably all-zero)."""
    weight = np.asarray(weight, np.float32)
    mu = np.asarray(mu, np.float32)
    var = np.asarray(var, np.float32)
    acd = np.asarray(acd, np.float32)
    labels = np.asarray(labels).astype(np.int64)
    nontrivial = np.asarray(nontrivial).astype(bool)

    sw = weight[labels]                                        # [B, D]
    diff = sw - mu
    density = np.exp(-(diff ** 2) / (2.0 * var))               # [B, D]
    total_density = np.maximum(acd.sum(axis=1), np.float32(1e-8))

    argmin_idx = acd.argmin(axis=1)                            # [B, D]
    kill = np.arange(C, dtype=np.int64)[None, :, None] == argmin_idx[:, None, :]
    nt = nontrivial & ~kill
    minv = (acd + (~nt) * np.float32(1000.0)).min(axis=1, keepdims=True)
    maxv = (acd - nt * np.float32(1000.0)).max(axis=1, keepdims=True)
    overly = (nt & (minv - maxv >= 0.2 * total_density[:, None, :])).astype(np.float32)

    confid = density[:, None, :] / total_density[:, None, :] * (1.0 - overly)

    dis = (
        (sw ** 2).sum(axis=1, keepdims=True)
        - 2.0 * sw @ weight.T
        + (weight ** 2).sum(axis=1)[None, :]
    )
    topkidx = np.argsort(dis, axis=1, kind="stable")[:, :TOPK]  # k smallest
    topk_w = weight[topkidx]                                    # [B, K, D]
    acd_det = np.exp(-((topk_w - mu[:, None, :]) ** 2) / (2.0 * var[:, None, :]))
    total_det = np.maximum(acd_det.sum(axis=1), np.float32(1e-8))
    confid = confid - density[:, None, :] / total_det[:, None, :] * overly

    return np.asarray(confid.mean(axis=-1).mean(), dtype=np.float32)


def _finish_host(weight, mu, var, labels, sums, global_min):
    """Combine per-core device partials into the final scalar."""
    S = sums[:, :D].astype(np.float64) + sums[:, D:].astype(np.float64)  # [B, D]
    td = np.maximum(S, 1e-8)

    # overly == 0 certificate: with all densities >= 0,
    # min2 <= S/(C-1) < 0.2*max(S, 1e-8) for C = 8192, so the overly
    # mask in the reference is identically zero.
    ok = bool(global_min >= 0.0)

    sw = np.asarray(weight, np.float32)[np.asarray(labels).astype(np.int64)]
    diff = sw.astype(np.float64) - np.asarray(mu, np.float64)
    density = np.exp(-(diff ** 2) / (2.0 * np.asarray(var, np.float64)))
    result = np.asarray((density / td).mean(), dtype=np.float32)
    return result, ok


def kernel(weight, mu, var, all_class_density, labels, nontrivial):
    acd = np.ascontiguousarray(np.asarray(all_class_density, dtype=np.float32))
    res = _run_device(acd).results
    sums = np.concatenate([r["out_sum"] for r in res], axis=0)   # [B, 512]
    # each core's last local row routed its taper tiles through DVE
    # fold-adds (out_part = per-partition-lane partials); fold the 128
    # lanes and add into that row's sums.
    for i, r in enumerate(res):
        sums[i * BS + BS - 1, :D] += r["out_part"].sum(axis=0, dtype=np.float32)
    global_min = min(float(r["out_min"].min()) for r in res)
    result, ok = _finish_host(weight, mu, var, labels, sums, global_min)
    if not ok or not bool(np.all(nontrivial)):
        return _reference_host(weight, mu, var, acd, labels, nontrivial)
    return result



# revision 2
# speedup vs baseline: 2.7412x; 2.7412x over previous
"""Trainium2 Bass kernel for nn_Density_Softmax (retrieval_knn).

Math
----
reference() computes, for B=32, C=8192, D=256:

    confid[b,c,d] = density[b,d]/total_density[b,d] * (1-overly[b,c,d])
                    - density[b,d]/total_det[b,d] * overly[b,c,d]
    out = confid.mean()

with overly[b,c,d] = (c != argmin_c acd[b,:,d]) & (min2 - min1 >= 0.2*total_density[b,d])
(min1/min2 = two smallest of acd[b,:,d]; nontrivial is all-ones by construction).

Since min2 <= (S - min1)/(C-1)  (a minimum is <= the mean), the condition
min2 - min1 >= 0.2*S is impossible when all densities are >= 0 (checked
exactly on host), so overly == 0 identically and

    out = mean_{b,d}( density[b,d] / max(sum_c acd[b,c,d], 1e-8) ).

The device work is therefore a single pure reduction over the 268 MB
all_class_density stream - a memory-roofline problem. To cut HBM traffic
4x, the host quantizes acd to fp8 (e4m3, round-to-nearest; values are in
[0,1) so this is exact to ~2^-9 relative) and the device sums fp8 with
f32 PSUM accumulation. Each per-(b,d) sum averages 8192 independent
rounding errors, and the final scalar averages 8192 such sums, so the
end-to-end error is ~5e-6 - far below the 2e-2 gate. A full exact host
fallback runs if the non-negativity certificate fails or nontrivial is
not all-ones.

Device layout (per core, 4 of 32 batch rows):
  - The fp8 image of acd[b] ([8192, 256] row-major) is viewed as
    [128, 16384]: partition p holds the 64 consecutive c-rows
    [64p, 64p+64) = 16 KB contiguous per partition (maximal DMA
    efficiency). Because the sum over c is commutative, this needs NO
    host-side transpose - it is a pure reshape of the fp8 byte image.
  - sum over c: PE DoubleRow fp8 matmul. lhsT = ones [128, 2, 1] fp8,
    rhs = tile chunk [128, 2, 512] fp8 -> psum [1, 512] f32, accumulated
    over all chunks of a batch row. DoubleRow contracts 256 elements per
    PSUM column per instruction at 0.5 cycles/row - 4x the reduction
    throughput of the fp32r path, keeping PE well under the DMA roofline.
  - PSUM column n accumulates d = n%256, c-parity n//256; host adds the
    two halves.
"""

import os

import ml_dtypes
import numpy as np

import concourse.bacc as bacc
import concourse.bass as bass
import concourse.tile as tile
from concourse import mybir
from concourse.bass_utils import run_bass_kernel_spmd

B, C, D = 32, 8192, 256
TOPK = 512
N_CORES = 8
BS = B // N_CORES  # batch rows per core

FREE_PER_B = C * D // 128  # fp8 bytes per partition per batch row (16384)

# SBUF tile free width in bytes per partition (per-DMA line = CHUNK bytes,
# contiguous). Must divide 16384 and be a multiple of 1024.
CHUNK = int(os.environ.get("DS_CHUNK", "4096"))
N_BUFS = int(os.environ.get("DS_BUFS", "8"))
DOUBLEROW = int(os.environ.get("DS_DOUBLEROW", "1"))

_STATE = {}


def _build_nc():
    f32 = mybir.dt.float32
    f8 = mybir.dt.float8e4
    u8 = mybir.dt.uint8
    assert FREE_PER_B % CHUNK == 0 and CHUNK % 1024 == 0
    n_tiles = FREE_PER_B // CHUNK  # per batch row
    k_per_tile = CHUNK // 1024  # DoubleRow matmuls per tile

    nc = bacc.Bacc("TRN2", target_bir_lowering=False, debug=False)
    acd8 = nc.dram_tensor("acd8", [BS, 128, FREE_PER_B], u8, kind="ExternalInput").ap()
    out_sum = nc.dram_tensor("out_sum", [BS, 512], f32, kind="ExternalOutput").ap()

    with tile.TileContext(nc) as tc:
        with (
            tc.tile_pool(name="big", bufs=N_BUFS) as big_pool,
            tc.tile_pool(name="souts", bufs=2) as sum_pool,
            tc.tile_pool(name="const", bufs=1) as const_pool,
            tc.tile_pool(name="psum", bufs=4, space="PSUM") as psum_pool,
        ):
            # all-ones fp8 weights; [128, 32] so the DoubleRow pair dim can
            # stride 16 bytes (HW requires the k-tile step % 16 == 0)
            ones = const_pool.tile([128, 32], f8, tag="ones")
            nc.vector.memset(ones[:], 1.0)
            ones_dr = ones[:].rearrange("p (i o) -> p i o", i=2)[:, :, 0:1]

            ps = {
                b: psum_pool.tile([1, 512], f32, name=f"ps{b}", tag="ps")
                for b in range(BS)
            }
            # round-robin the rows so 4 PSUM accumulation chains stay live
            # and the PE always has a DMA-complete tile to chew on
            for t in range(n_tiles):
                for b in range(BS):
                    big = big_pool.tile([128, CHUNK], u8, tag="big")
                    nc.sync.dma_start(
                        big[:], acd8[b, :, t * CHUNK : (t + 1) * CHUNK]
                    )
                    if DOUBLEROW:
                        for k in range(k_per_tile):
                            rhs = (
                                big[:, k * 1024 : (k + 1) * 1024]
                                .bitcast(f8)
                                .rearrange("p (i n) -> p i n", i=2)
                            )
                            nc.tensor.matmul(
                                ps[b][:],
                                ones_dr,
                                rhs,
                                start=(t == 0 and k == 0),
                                stop=(t == n_tiles - 1 and k == k_per_tile - 1),
                                perf_mode=mybir.MatmulPerfMode.DoubleRow,
                            )
                    else:
                        for k in range(CHUNK // 512):
                            rhs = big[:, k * 512 : (k + 1) * 512].bitcast(f8)
                            nc.tensor.matmul(
                                ps[b][:],
                                ones[:, 0:1],
                                rhs,
                                start=(t == 0 and k == 0),
                                stop=(t == n_tiles - 1 and k == CHUNK // 512 - 1),
                            )
                    if t == n_tiles - 1:
                        sout = sum_pool.tile([1, 512], f32)
                        nc.scalar.copy(sout[:], ps[b][:])
                        nc.sync.dma_start(out_sum[b : b + 1, :], sout[:])
    nc.compile()
    return nc


def _get_nc():
    if "nc" not in _STATE:
        _STATE["nc"] = _build_nc()
    return _STATE["nc"]


def _get_runner():
    """Sharded executor built once. The input shards are device_put and
    blocked-on BEFORE dispatch, so all 8 cores start aligned and the
    kernel's HBM reads don't contend with input-upload writes."""
    if "runner" in _STATE:
        return _STATE["runner"]
    import jax
    import numpy as _np
    from jax.experimental.shard_map import shard_map
    from jax.sharding import Mesh, NamedSharding, PartitionSpec

    from concourse import bass2jax, mybir as _mybir

    bass2jax.install_neuronx_cc_hook()
    nc = _get_nc()

    partition_name = nc.partition_id_tensor.name if nc.partition_id_tensor else None
    in_names, out_names, out_avals, zero_outs = [], [], [], []
    for alloc in nc.m.functions[0].allocations:
        if not isinstance(alloc, _mybir.MemoryLocationSet):
            continue
        name = alloc.memorylocations[0].name
        if alloc.kind == "ExternalInput":
            if name != partition_name:
                in_names.append(name)
        elif alloc.kind == "ExternalOutput":
            out_names.append(name)
            shape = tuple(alloc.tensor_shape)
            dtype = _mybir.dt.np(alloc.dtype)
            out_avals.append(jax.core.ShapedArray(shape, dtype))
            zero_outs.append(_np.zeros(shape, dtype))
    n_params = len(in_names)
    n_outs = len(out_avals)
    all_in_names = list(in_names) + list(out_names)
    if partition_name is not None:
        all_in_names.append(partition_name)
    donate = tuple(range(n_params, n_params + n_outs))

    def _body(*args):
        operands = list(args)
        if partition_name is not None:
            operands.append(bass2jax.partition_id_tensor())
        outs = bass2jax._bass_exec_p.bind(
            *operands,
            out_avals=tuple(out_avals),
            in_names=tuple(all_in_names),
            out_names=tuple(out_names),
            lowering_input_output_aliases=(),
            sim_require_finite=True,
            sim_require_nnan=True,
            nc=nc,
        )
        return tuple(outs)

    devices = jax.devices()[:N_CORES]
    mesh = Mesh(_np.asarray(devices), ("core",))
    spec = NamedSharding(mesh, PartitionSpec("core"))
    in_specs = (PartitionSpec("core"),) * (n_params + n_outs)
    out_specs = (PartitionSpec("core"),) * n_outs
    sharded = jax.jit(
        shard_map(_body, mesh=mesh, in_specs=in_specs, out_specs=out_specs,
                  check_rep=False),
        donate_argnums=donate,
        keep_unused=True,
    )

    def run(in_map_global):
        import jax as _jax

        args = []
        for name in in_names:
            args.append(_jax.device_put(in_map_global[name], spec))
        for z in zero_outs:
            gz = _np.zeros((N_CORES * z.shape[0], *z.shape[1:]), z.dtype)
            args.append(_jax.device_put(gz, spec))
        for a in args:
            a.block_until_ready()
        outs = sharded(*args)
        outs = [_np.asarray(o) for o in outs]
        return [
            {
                name: outs[i].reshape(N_CORES, *out_avals[i].shape)[c]
                for i, name in enumerate(out_names)
            }
            for c in range(N_CORES)
        ]

    _STATE["runner"] = run
    return run


class _Res:
    def __init__(self, results):
        self.results = results


def _pack_fp8(acd):
    """f32 [B, C, D] -> fp8 byte image [B, 128, FREE_PER_B] (pure reshape:
    partition p of row b holds c-rows [64p, 64p+64))."""
    a8 = acd.astype(ml_dtypes.float8_e4m3)
    return a8.view(np.uint8).reshape(B, 128, FREE_PER_B)


def _run_device(acd, **kw):
    packed = _pack_fp8(np.ascontiguousarray(np.asarray(acd, np.float32)))
    try:
        return _Res(_get_runner()({"acd8": packed}))
    except Exception:
        # robust fallback: stock SPMD path (handles native-NRT and axon)
        nc = _get_nc()
        in_maps = [
            {"acd8": packed[i * BS : (i + 1) * BS]} for i in range(N_CORES)
        ]
        return run_bass_kernel_spmd(nc, in_maps, list(range(N_CORES)))


def _reference_host(weight, mu, var, acd, labels, nontrivial):
    """Exact numpy mirror of reference.py (fallback; not used for graded
    inputs, where the overly mask is provably all-zero)."""
    weight = np.asarray(weight, np.float32)
    mu = np.asarray(mu, np.float32)
    var = np.asarray(var, np.float32)
    acd = np.asarray(acd, np.float32)
    labels = np.asarray(labels).astype(np.int64)
    nontrivial = np.asarray(nontrivial).astype(bool)

    sw = weight[labels]                                        # [B, D]
    diff = sw - mu
    density = np.exp(-(diff ** 2) / (2.0 * var))               # [B, D]
    total_density = np.maximum(acd.sum(axis=1), np.float32(1e-8))

    argmin_idx = acd.argmin(axis=1)                            # [B, D]
    kill = np.arange(C, dtype=np.int64)[None, :, None] == argmin_idx[:, None, :]
    nt = nontrivial & ~kill
    minv = (acd + (~nt) * np.float32(1000.0)).min(axis=1, keepdims=True)
    maxv = (acd - nt * np.float32(1000.0)).max(axis=1, keepdims=True)
    overly = (nt & (minv - maxv >= 0.2 * total_density[:, None, :])).astype(np.float32)

    confid = density[:, None, :] / total_density[:, None, :] * (1.0 - overly)

    dis = (
        (sw ** 2).sum(axis=1, keepdims=True)
        - 2.0 * sw @ weight.T
        + (weight ** 2).sum(axis=1)[None, :]
    )
    topkidx = np.argsort(dis, axis=1, kind="stable")[:, :TOPK]  # k smallest
    topk_w = weight[topkidx]                                    # [B, K, D]
    acd_det = np.exp(-((topk_w - mu[:, None, :]) ** 2) / (2.0 * var[:, None, :]))
    total_det = np.maximum(acd_det.sum(axis=1), np.float32(1e-8))
    confid = confid - density[:, None, :] / total_det[:, None, :] * overly

    return np.asarray(confid.mean(axis=-1).mean(), dtype=np.float32)


def _finish_host(weight, mu, var, labels, sums, global_min):
    """Combine per-core device partials into the final scalar."""
    S = sums[:, :D].astype(np.float64) + sums[:, D:].astype(np.float64)  # [B, D]
    td = np.maximum(S, 1e-8)

    # overly == 0 certificate: with all densities >= 0,
    # min2 <= S/(C-1) < 0.2*max(S, 1e-8) for C = 8192, so the overly
    # mask in the reference is identically zero.
    ok = bool(global_min >= 0.0)

    sw = np.asarray(weight, np.float32)[np.asarray(labels).astype(np.int64)]
    diff = sw.astype(np.float64) - np.asarray(mu, np.float64)
    density = np.exp(-(diff ** 2) / (2.0 * np.asarray(var, np.float64)))
    result = np.asarray((density / td).mean(), dtype=np.float32)
    return result, ok


def kernel(weight, mu, var, all_class_density, labels, nontrivial):
    acd = np.ascontiguousarray(np.asarray(all_class_density, dtype=np.float32))
    res = _run_device(acd).results
    sums = np.concatenate([r["out_sum"] for r in res], axis=0)   # [B, 512]
    global_min = float(acd.min())
    result, ok = _finish_host(weight, mu, var, labels, sums, global_min)
    if not ok or not bool(np.all(nontrivial)):
        return _reference_host(weight, mu, var, acd, labels, nontrivial)
    return result
